# revision 1
# baseline (speedup 1.0000x reference)
"""Trainium2 Bass kernel for suffix-softmax attention visualization.

Computes, for hidden_states [S, B, H], W [H, 1], b [1]:
    s[t, b]   = sum_h hidden_states[t, b, h] * W[h, 0] + b[0]
    out[t, b] = exp(s[t, b]) / sum_{t' >= t} exp(s[t', b])     (suffix softmax)
returned as [S, B, 1] f32.

The softmax ratio is shift-invariant, so the scalar bias b cancels exactly
and is not needed on device. The scores are N(0, 1)-scaled by construction
(W drawn as randn/sqrt(H)), so exp() needs no max-subtraction in f32.

Sharding: data-parallel over the batch axis — 8 NeuronCores, 8 batch
columns each. Per core:
  - 32 blocks of [128 s, 8 b, 512 h] stream from HBM (2 MiB DMAs with
    16 KiB contiguous rows, ~full HBM bandwidth); the first blocks are
    split into smaller chunks so compute starts sooner;
  - DVE scalar_tensor_tensor fuses the W-multiply and the h-reduction in
    a single 1x pass per (block, b) — the DVE is the pacing engine;
  - ACT computes exp per block (hides the activation-table load);
  - the suffix sum uses two half-tile lower-triangular matmuls on the PE
    (within-block scan), Hillis-Steele shifted adds over the 32 block
    totals, and a K=1 ones-matmul to broadcast the cross-block offsets;
  - DVE divides (2-ULP approx reciprocal + multiply) in four chunks so
    the output DMAs overlap the remaining compute; the host reassembles
    the eight [4096, 8] per-core outputs.
"""

import numpy as np

import concourse.bacc as bacc
import concourse.mybir as mybir
import concourse.tile as tile
from concourse import bass_utils

P = 128
S = 4096
B = 64
H = 512
N_CORES = 8
BC = B // N_CORES  # batch columns per core
NBLK = S // P
C = NBLK * BC  # score-tile columns, c = blk*BC + b

def build_program(hs_bufs=8, gp_cols=0, approx_recip=True, block_scan=False, use_amr=False, Bc=BC):
    """Build the per-core Bass program.

    Inputs : hs [S, Bc, H] f32, wb [128, H] f32 (W broadcast),
             tri [128, 128] f32 lower-triangular ones, onesq [128, 128] ones.
    Output : out [S, Bc] f32.
    """
    assert S % P == 0
    NBLK = S // P
    assert NBLK <= 32, "Hillis-Steele pad sized for <= 32 blocks"
    C = NBLK * Bc

    nc = bacc.Bacc("TRN2", target_bir_lowering=False, debug=False)
    hs = nc.dram_tensor("hs", [S, Bc, H], mybir.dt.float32, kind="ExternalInput")
    wb = nc.dram_tensor("wb", [P, H], mybir.dt.float32, kind="ExternalInput")
    tri = nc.dram_tensor("tri", [P, P], mybir.dt.float32, kind="ExternalInput")
    onesq = nc.dram_tensor("onesq", [P, P], mybir.dt.float32, kind="ExternalInput")
    out = nc.dram_tensor("out", [S, Bc], mybir.dt.float32, kind="ExternalOutput")

    with tile.TileContext(nc) as tc:
        with (
            tc.tile_pool(name="hsp", bufs=hs_bufs) as hsp,
            tc.tile_pool(name="consts", bufs=1) as consts,
            tc.tile_pool(name="work", bufs=1) as work,
            tc.tile_pool(name="prodp", bufs=3) as prodp,
            tc.tile_pool(name="psum", bufs=1, space="PSUM") as psum,
        ):
            # Stream DMAs go on the SP HWDGE ring; constants ride the ACT
            # ring so they don't delay the first hs block.
            hs_ap = hs.ap()
            hs_tiles = []
            split_plan = {0: 1, 1: 1, 2: 2, 3: 2, 4: 4}  # blk -> cols per DMA chunk
            for blk in range(NBLK):
                hst = hsp.tile([P, Bc, H], mybir.dt.float32)
                rows = hs_ap[blk * P : (blk + 1) * P, :, :]
                qb = min(split_plan.get(blk, Bc), Bc)
                for q in range(0, Bc, qb):
                    nc.sync.dma_start(
                        out=hst[:, q : q + qb, :], in_=rows[:, q : q + qb, :]
                    )
                hs_tiles.append(hst)

            wb_t = consts.tile([P, H], mybir.dt.float32)
            nc.scalar.dma_start(out=wb_t, in_=wb.ap())
            tri_t = consts.tile([P, P], mybir.dt.float32)
            nc.scalar.dma_start(out=tri_t, in_=tri.ap())
            onesq_t = consts.tile([P, P], mybir.dt.float32)
            nc.scalar.dma_start(out=onesq_t, in_=onesq.ap())

            s_col = work.tile([P, C], mybir.dt.float32)
            e_t = work.tile([P, C], mybir.dt.float32)
            dummy = work.tile([P, 1], mybir.dt.float32)
            dummy2 = work.tile([P, 1], mybir.dt.float32)
            scan_ps = psum.tile([P, C], mybir.dt.float32)

            for blk in range(NBLK):
                hst = hs_tiles[blk]
                for b in range(Bc):
                    c = blk * Bc + b
                    if b < gp_cols:
                        prod = prodp.tile([P, H], mybir.dt.float32)
                        nc.gpsimd.tensor_tensor(
                            prod, hst[:, b, :], wb_t, op=mybir.AluOpType.mult
                        )
                        nc.scalar.activation(
                            dummy2.broadcast_to((P, H)),
                            prod,
                            mybir.ActivationFunctionType.Copy,
                            accum_out=s_col[:, c : c + 1],
                        )
                    elif use_amr:
                        nc.vector.affine_mul_reduce(
                            out=dummy.broadcast_to((P, H)),
                            accum_out=s_col[:, c : c + 1],
                            in0=hst[:, b, :],
                            in1=wb_t,
                            scale=1.0,
                            bias=0.0,
                        )
                    else:
                        nc.vector.scalar_tensor_tensor(
                            out=dummy.broadcast_to((P, H)),
                            in0=hst[:, b, :],
                            scalar=1.0,
                            in1=wb_t,
                            op0=mybir.AluOpType.mult,
                            op1=mybir.AluOpType.mult,
                            accum_out=s_col[:, c : c + 1],
                        )
                lo, hi = blk * Bc, (blk + 1) * Bc
                nc.scalar.activation(
                    e_t[:, lo:hi], s_col[:, lo:hi], mybir.ActivationFunctionType.Exp
                )
                if block_scan:
                    # Within-block inclusive suffix sums:
                    # scan_ps[m, c] = sum_{k>=m} e[k, c]
                    nc.tensor.matmul(
                        scan_ps[:, lo:hi], tri_t, e_t[:, lo:hi], start=True, stop=True
                    )

                if not block_scan and blk == NBLK // 2 - 1:
                    nc.tensor.matmul(
                        scan_ps[:, : C // 2],
                        tri_t,
                        e_t[:, : C // 2],
                        start=True,
                        stop=True,
                    )

            # Block totals broadcast to every partition in one matmul:
            # totb_ps[m, c] = sum_k 1 * e[k, c]  (same value for all m)
            totb_ps = psum.tile([P, C], mybir.dt.float32)
            nc.tensor.matmul(totb_ps, onesq_t, e_t, start=True, stop=True)

            if not block_scan:
                nc.tensor.matmul(
                    scan_ps[:, C // 2 :], tri_t, e_t[:, C // 2 :], start=True, stop=True
                )

            # Cross-block exclusive suffix offsets, computed broadcast on all
            # partitions (Hillis-Steele over the 32 block totals).
            TLEN = (NBLK + 1) * Bc
            PAD = 16 * Bc
            t0 = work.tile([P, TLEN + PAD], mybir.dt.float32)
            t1 = work.tile([P, TLEN + PAD], mybir.dt.float32)
            # only the shifted-read pads need zeroing; the data region is
            # fully written by the copy / first add
            nc.vector.memset(t0[:, C:], 0.0)
            nc.vector.memset(t1[:, TLEN:], 0.0)
            nc.vector.tensor_copy(t0[:, 0:C], totb_ps)
            src, dst = t0, t1
            d = 1
            while d < NBLK:
                nc.vector.tensor_add(
                    dst[:, 0:TLEN],
                    src[:, 0:TLEN],
                    src[:, d * Bc : d * Bc + TLEN],
                )
                src, dst = dst, src
                d *= 2
            # src[p, blk*Bc + b] = sum_{blk' >= blk} totals[blk', b]
            # offsets for blk = value at blk+1  (exclusive suffix)
            bsb = src[:, Bc : Bc + C]

            # selected = e / S, in two halves so the out-DMA overlaps compute.
            ssum = work.tile([P, C], mybir.dt.float32)
            rec = work.tile([P, C], mybir.dt.float32)
            scr = work.tile([P, C // 2], mybir.dt.float32)
            sel = work.tile([P, C], mybir.dt.float32)
            out_ap = out.ap().rearrange("(blk p) b -> p blk b", p=P)
            nparts = min(2, NBLK)
            pb = NBLK // nparts
            for h in range(nparts):
                lo, hi = h * (C // nparts), (h + 1) * (C // nparts)
                nc.vector.tensor_add(
                    ssum[:, lo:hi], bsb[:, lo:hi], scan_ps[:, lo:hi]
                )
                if approx_recip == "divide":
                    nc.vector.tensor_tensor(
                        sel[:, lo:hi],
                        e_t[:, lo:hi],
                        ssum[:, lo:hi],
                        op=mybir.AluOpType.divide,
                    )
                elif approx_recip:
                    nc.vector.reciprocal_approx_accurate(
                        rec[:, lo:hi], ssum[:, lo:hi], scr[:, : hi - lo]
                    )
                    nc.vector.tensor_mul(sel[:, lo:hi], e_t[:, lo:hi], rec[:, lo:hi])
                else:
                    nc.vector.reciprocal(rec[:, lo:hi], ssum[:, lo:hi])
                    nc.vector.tensor_mul(sel[:, lo:hi], e_t[:, lo:hi], rec[:, lo:hi])
                sel_ap = sel[:, lo:hi].rearrange("p (blk b) -> p blk b", b=Bc)
                nc.sync.dma_start(
                    out=out_ap[:, h * pb : (h + 1) * pb, :],
                    in_=sel_ap,
                )

    nc.compile()
    return nc


_PROGRAM = None


def _get_program():
    global _PROGRAM
    if _PROGRAM is None:
        _PROGRAM = build_program()
    return _PROGRAM


def make_in_maps(hidden_states, W):
    hidden_states = np.asarray(hidden_states, dtype=np.float32)
    W = np.asarray(W, dtype=np.float32)
    wb = np.ascontiguousarray(np.broadcast_to(W[:, 0][None, :], (P, H)))
    tri = np.tril(np.ones((P, P), dtype=np.float32))
    onesq = np.ones((P, P), dtype=np.float32)
    in_maps = []
    for c in range(N_CORES):
        hs_c = np.ascontiguousarray(hidden_states[:, c * BC : (c + 1) * BC, :])
        in_maps.append({"hs": hs_c, "wb": wb, "tri": tri, "onesq": onesq})
    return in_maps


def assemble_output(results):
    cols = [results[c]["out"] for c in range(N_CORES)]
    return np.concatenate(cols, axis=1)[..., None].astype(np.float32)


def kernel(hidden_states, W, b):
    nc = _get_program()
    in_maps = make_in_maps(hidden_states, W)
    res = bass_utils.run_bass_kernel_spmd(nc, in_maps, core_ids=list(range(N_CORES)))
    return assemble_output(res.results)



# revision 4
# speedup vs baseline: 1.1522x; 1.1522x over previous
"""Trainium2 Bass kernel for suffix-softmax attention visualization.

Computes, for hidden_states [S, B, H], W [H, 1], b [1]:
    s[t, b]   = sum_h hidden_states[t, b, h] * W[h, 0] + b[0]
    out[t, b] = exp(s[t, b]) / sum_{t' >= t} exp(s[t', b])     (suffix softmax)
returned as [S, B, 1] f32.  The softmax ratio is shift-invariant so b cancels.

Sharding: data-parallel over batch — 8 NeuronCores, 8 batch columns each.

v2 design (memory-regime; per-core floor = 64 MiB / ~358 GB/s ~= 187 us):
  - hs blocks [128, 8, 512] stream via SWDGE (gpsimd) DMA with an inline
    f32->fp16 cast: HBM reads stay f32 (unavoidable) but SBUF tiles are fp16,
    which unlocks the DVE 2x_1p perf mode for the multiply.
  - One DVE tensor_tensor multiply per block (fp16, 2x) forms prod = hs * w
    for all 8 columns in a single instruction.
  - The h-reduction is split across engines: ACT_COLS columns via ScalarE
    activation(Copy, accum_out=...) at 1 elem/cycle, the rest via one grouped
    VectorE tensor_reduce over a 3D AP ([128, n, 512] -> [128, n]).
  - Blocks are processed in REVERSE seq order with a running esum[p, b] =
    sum of e over already-processed (later) blocks.  Per block the suffix
    denominator = tri @ e_blk + ones @ esum lands in PSUM via two tiny PE
    matmuls, so there is no global scan epilogue at all; out chunks DMA
    back while earlier blocks still stream in.
  - Output is written [128 p, 32 j, 8 b] (j = reversed block index,
    contiguous per partition); the host unscrambles to [4096, 8].
"""

import numpy as np

import concourse.bacc as bacc
import concourse.mybir as mybir
import concourse.tile as tile
from concourse import bass_utils

P = 128
S = 4096
B = 64
H = 512
N_CORES = 8
BC = B // N_CORES  # batch columns per core
NBLK = S // P      # 32 seq blocks per core
C = NBLK * BC      # output columns per partition, c = j*BC + b

F32 = mybir.dt.float32
F16 = mybir.dt.float16


def build_program(act_cols=5, hs_bufs=12, out_chunks=4):
    """Per-core program.

    Inputs : hs [S, BC, H] f32, wrep [P, BC*H] fp16 (w tiled BC times),
             tri [P, P] f32 lower-triangular ones, onesq [P, P] f32 ones.
    Output : out [P, C] f32, out[p, j*BC+b] = selected[(NBLK-1-j)*P + p, b].
    """
    nc = bacc.Bacc("TRN2", target_bir_lowering=False, debug=False)
    hs = nc.dram_tensor("hs", [S, BC, H], F32, kind="ExternalInput")
    wrep = nc.dram_tensor("wrep", [P, BC * H], F16, kind="ExternalInput")
    tri = nc.dram_tensor("tri", [P, P], F32, kind="ExternalInput")
    onesq = nc.dram_tensor("onesq", [P, P], F32, kind="ExternalInput")
    out = nc.dram_tensor("out", [P, C], F32, kind="ExternalOutput")

    with tile.TileContext(nc) as tc:
        with (
            tc.tile_pool(name="hsp", bufs=hs_bufs) as hsp,
            tc.tile_pool(name="prodp", bufs=3) as prodp,
            tc.tile_pool(name="consts", bufs=1) as consts,
            tc.tile_pool(name="scp", bufs=4) as scp,
            tc.tile_pool(name="ep", bufs=4) as ep,
            tc.tile_pool(name="work", bufs=1) as work,
            tc.tile_pool(name="psum", bufs=4, space="PSUM") as psum,
        ):
            # Constants ride the ACT HWDGE ring so they don't delay the
            # gpsimd stream queue.
            wrep_t = consts.tile([P, BC * H], F16)
            nc.scalar.dma_start(out=wrep_t, in_=wrep.ap())
            tri_t = consts.tile([P, P], F32)
            nc.scalar.dma_start(out=tri_t, in_=tri.ap())
            onesq_t = consts.tile([P, P], F32)
            nc.scalar.dma_start(out=onesq_t, in_=onesq.ap())

            # Stream all hs blocks in reverse seq order via SWDGE cast DMAs.
            hs_ap = hs.ap()
            hs_tiles = []
            for j in range(NBLK):
                k = NBLK - 1 - j
                hst = hsp.tile([P, BC, H], F16)
                rows = hs_ap[k * P : (k + 1) * P, :, :]
                if j < 2:
                    # smaller first transfers so compute ramps sooner
                    half = BC // 2
                    nc.gpsimd.dma_start(out=hst[:, :half, :], in_=rows[:, :half, :])
                    nc.gpsimd.dma_start(out=hst[:, half:, :], in_=rows[:, half:, :])
                else:
                    nc.gpsimd.dma_start(out=hst, in_=rows)
                hs_tiles.append(hst)

            wrep_v = wrep_t.rearrange("p (b h) -> p b h", h=H)
            dummy = work.tile([P, 1], F32)
            esum = [
                work.tile([P, BC], F32, name="esum0"),
                work.tile([P, BC], F32, name="esum1"),
            ]
            nc.vector.memset(esum[0], 0.0)
            sel = work.tile([P, C], F32)
            out_ap = out.ap()

            blk_per_chunk = NBLK // out_chunks
            for j in range(NBLK):
                hst = hs_tiles[j]
                prod = prodp.tile([P, BC, H], F16)
                nc.vector.tensor_tensor(prod, hst, wrep_v, op=mybir.AluOpType.mult)

                s_col = scp.tile([P, BC], F32)
                for b in range(act_cols):
                    nc.scalar.activation(
                        dummy.broadcast_to((P, H)),
                        prod[:, b, :],
                        mybir.ActivationFunctionType.Copy,
                        accum_out=s_col[:, b : b + 1],
                    )
                if act_cols < BC:
                    nc.vector.reduce_sum(
                        out=s_col[:, act_cols:],
                        in_=prod[:, act_cols:, :],
                        axis=mybir.AxisListType.X,
                    )

                e_t = ep.tile([P, BC], F32)
                nc.scalar.activation(e_t, s_col, mybir.ActivationFunctionType.Exp)

                # denom[m, b] = sum_{p>=m} e[p, b] + sum_p esum[p, b]
                ps = psum.tile([P, BC], F32)
                nc.tensor.matmul(ps, tri_t, e_t, start=True, stop=False)
                nc.tensor.matmul(ps, onesq_t, esum[j % 2], start=False, stop=True)

                nc.vector.tensor_add(esum[(j + 1) % 2], esum[j % 2], e_t)
                rec = ep.tile([P, BC], F32, name=f"rec{j}")
                nc.vector.reciprocal(rec, ps)
                nc.vector.tensor_mul(sel[:, j * BC : (j + 1) * BC], e_t, rec)

                if (j + 1) % blk_per_chunk == 0:
                    g = j // blk_per_chunk
                    lo, hi = g * blk_per_chunk * BC, (g + 1) * blk_per_chunk * BC
                    nc.sync.dma_start(out=out_ap[:, lo:hi], in_=sel[:, lo:hi])

    nc.compile()
    return nc


_PROGRAM = None


def _get_program():
    global _PROGRAM
    if _PROGRAM is None:
        _PROGRAM = build_program()
    return _PROGRAM


def make_in_maps(hidden_states, W):
    hidden_states = np.asarray(hidden_states, dtype=np.float32)
    w16 = np.asarray(W, dtype=np.float32)[:, 0].astype(np.float16)
    wrep = np.ascontiguousarray(np.tile(w16[None, :], (P, BC)))
    tri = np.tril(np.ones((P, P), dtype=np.float32))
    onesq = np.ones((P, P), dtype=np.float32)
    in_maps = []
    for c in range(N_CORES):
        hs_c = np.ascontiguousarray(hidden_states[:, c * BC : (c + 1) * BC, :])
        in_maps.append({"hs": hs_c, "wrep": wrep, "tri": tri, "onesq": onesq})
    return in_maps


def assemble_output(results):
    cols = []
    for c in range(N_CORES):
        oc = results[c]["out"]  # [P, C], col = j*BC + b, j = reversed block
        full = oc.reshape(P, NBLK, BC)[:, ::-1, :].transpose(1, 0, 2).reshape(S, BC)
        cols.append(full)
    return np.concatenate(cols, axis=1)[..., None].astype(np.float32)


def kernel(hidden_states, W, b):
    nc = _get_program()
    in_maps = make_in_maps(hidden_states, W)
    res = bass_utils.run_bass_kernel_spmd(nc, in_maps, core_ids=list(range(N_CORES)))
    return assemble_output(res.results)
